# revision 9
# baseline (speedup 1.0000x reference)
"""Trainium2 Bass kernel for nn_ContiguousMatch.

Math (per batch row b):
    v_match[b,l] = sum_a x[b,l,a] * v[l,a]            # [B, 40]
    v_score[b]   = sum_l cumprod_l(v_match[b,:])       # [B]
    j_score[b]   = sum_{l,a} x[b,l,a] * j[l,a]         # [B]
    out[b] = (v_score[b], j_score[b])                  # [B, 2]

Strategy (pure data parallel over 8 cores, B_shard = 16384 per core):
  All heavy math runs on the TensorEngine in bf16.  x tiles [128b, 840]
  stream in through a casting SWDGE DMA (fp32 HBM -> bf16 SBUF, zero
  compute cost), get transposed on-chip by PE into [120k, 512b] chunks
  (7 chunks cover the 840 = 40*21 feature axis), are copied PSUM->SBUF
  by DVE/ACT, then a block-diagonal germline matrix W [840, 41] (col l
  holds v[l,:] in rows l*21..l*21+20, col 40 holds j.flatten()) reduces
  them in 7 accumulating matmuls to S = [41, 512] fp32: rows 0..39 =
  v_match^T, row 40 = j_score (taken exactly from here).
  The cumprod-scan runs in log space: Ln on ACT with a constant scale
  e^{-c} (keeps bf16 logs centered near 0), cumulative sum as a matmul
  with a triangular ones matrix, Exp on ACT with a per-position bias
  (m+1)*c undoing the shift, and the sum over l as a ones-vector matmul.
  Results land interleaved in SBUF and stream out as one contiguous 4KB
  DMA per 512 rows.
"""

import numpy as np
import ml_dtypes
from contextlib import ExitStack

import concourse.bass as bass
import concourse.bacc as bacc
import concourse.tile as tile
from concourse import mybir
from concourse.bass_utils import run_bass_kernel_spmd

F32 = mybir.dt.float32
BF16 = mybir.dt.bfloat16
BF16_NP = ml_dtypes.bfloat16

N_CORES = 8
B_FULL = 131072
L, A = 40, 21
KA = L * A            # 840
NCHUNK = 7
KC = KA // NCHUNK     # 120
UNIT_B = 512          # batch rows handled per unit
NQ = UNIT_B // 128    # 4 sub-tiles of 128 rows

# log-space centering constants (numerical conditioning only — the shift is
# exactly undone by the Exp bias, so correctness holds for any input stats)
C_V = 1.6582280766542983  # ~ log(E[v_match]) = log(21/4)


def _np_consts():
    ident = np.eye(128, dtype=BF16_NP)
    # W col 0 = j, cols 1..40 = v one-hot blocks (j first so j_score lands
    # on PSUM partition 0, which keeps the extraction copy 32-aligned)
    mask = np.zeros((KC, NCHUNK, 41), BF16_NP)
    for c in range(NCHUNK):
        for p in range(KC):
            l = (c * KC + p) // A
            mask[p, c, 1 + l] = 1.0
    # cum[m] = sum of log rows 1..m+1 (row 0 = j row is skipped)
    tri = np.zeros((41, L), BF16_NP)
    for m in range(L):
        tri[1: m + 2, m] = 1.0
    colv = np.ones((L, 1), BF16_NP)
    expbias = ((np.arange(L, dtype=np.float64) + 1.0) * C_V).astype(np.float32)
    expbias = expbias.reshape(L, 1)
    return ident, mask, tri, colv, expbias


def emit_kernel(ctx: ExitStack, tc: tile.TileContext,
                x_ap, v_ap, j_ap, out_ap):
    """Emit the per-core program. x_ap [B,L,A], v/j [L,A], out [B,2]."""
    nc = tc.nc
    B = x_ap.shape[0]
    assert B % UNIT_B == 0
    n_units = B // UNIT_B

    ident_np, mask_np, tri_np, colv_np, expbias_np = _np_consts()
    ident_d = nc.inline_tensor(ident_np, name="c_ident")
    mask_d = nc.inline_tensor(mask_np, name="c_mask")
    tri_d = nc.inline_tensor(tri_np, name="c_tri")
    colv_d = nc.inline_tensor(colv_np, name="c_colv")
    expbias_d = nc.inline_tensor(expbias_np, name="c_expbias")

    singles = ctx.enter_context(tc.tile_pool(name="singles", bufs=1))
    xpool = ctx.enter_context(tc.tile_pool(name="xpool", bufs=3))
    xtpool = ctx.enter_context(tc.tile_pool(name="xtpool", bufs=4))
    lpool = ctx.enter_context(tc.tile_pool(name="lpool", bufs=2))
    epool = ctx.enter_context(tc.tile_pool(name="epool", bufs=2))
    stpool = ctx.enter_context(tc.tile_pool(name="stpool", bufs=3))
    ps_chunk = ctx.enter_context(tc.tile_pool(name="ps_chunk", bufs=3, space="PSUM"))
    ps_s1 = ctx.enter_context(tc.tile_pool(name="ps_s1", bufs=2, space="PSUM"))
    ps_cum = ctx.enter_context(tc.tile_pool(name="ps_cum", bufs=1, space="PSUM"))
    ps_vj = ctx.enter_context(tc.tile_pool(name="ps_vj", bufs=2, space="PSUM"))

    # ---- constants into SBUF ----
    ident_sb = singles.tile([128, 128], BF16)
    nc.sync.dma_start(out=ident_sb, in_=ident_d.ap())
    # mask goes through a DVE staging copy so the broadcast-multiply below
    # (TensorScalarPtr encoding: single wait slot) needs no cross-engine waits
    mask_stg = singles.tile([KC, NCHUNK, 41], BF16)
    nc.sync.dma_start(out=mask_stg, in_=mask_d.ap())
    mask_sb = singles.tile([KC, NCHUNK, 41], BF16)
    nc.vector.tensor_copy(out=mask_sb, in_=mask_stg)
    tri_sb = singles.tile([41, L], BF16)
    nc.sync.dma_start(out=tri_sb, in_=tri_d.ap())
    colv_sb = singles.tile([L, 1], BF16)
    nc.sync.dma_start(out=colv_sb, in_=colv_d.ap())
    expbias_sb = singles.tile([L, 1], F32)
    nc.sync.dma_start(out=expbias_sb, in_=expbias_d.ap())

    # ---- germline -> W [KC, NCHUNK, 41] (bf16) ----
    # load v/j flattened as [7, 120] (partition = chunk), PE-transpose to [120, 7]
    vg_sb = singles.tile([NCHUNK, KC], BF16)
    nc.gpsimd.dma_start(out=vg_sb,
                        in_=v_ap.flatten().rearrange("(c k) -> c k", c=NCHUNK))
    jg_sb = singles.tile([NCHUNK, KC], BF16)
    nc.gpsimd.dma_start(out=jg_sb,
                        in_=j_ap.flatten().rearrange("(c k) -> c k", c=NCHUNK))

    v_col = singles.tile([KC, NCHUNK], F32)
    j_col = singles.tile([KC, NCHUNK], F32)
    for src, dst in ((vg_sb, v_col), (jg_sb, j_col)):
        tr_ps = ps_chunk.tile([KC, UNIT_B], BF16, tag="ps_chunk")
        nc.tensor.transpose(tr_ps[:, :NCHUNK], src, ident_sb[:NCHUNK, :NCHUNK])
        nc.vector.tensor_copy(out=dst, in_=tr_ps[:, :NCHUNK])

    w_sb = singles.tile([KC, NCHUNK, 41], BF16)
    nc.vector.tensor_mul(w_sb[:, :, 1:41], mask_sb[:, :, 1:41],
                         v_col.to_broadcast([KC, NCHUNK, L]))
    nc.vector.tensor_copy(out=w_sb[:, :, 0], in_=j_col)

    # x viewed as [u, p, q, 840]
    x_view = x_ap.rearrange("(u q p) l a -> u p q (l a)", q=NQ, p=128)
    out_view = out_ap.rearrange("(u f) c -> u (f c)", f=UNIT_B)

    ln_scale = float(np.exp(-C_V))

    for u in range(n_units):
        xb = xpool.tile([128, NQ, KA], BF16, tag="xb")
        nc.gpsimd.dma_start(out=xb, in_=x_view[u])  # casting DMA fp32->bf16

        s1 = ps_s1.tile([41, UNIT_B], F32, tag="s1")
        for c in range(NCHUNK):
            ps = ps_chunk.tile([KC, UNIT_B], BF16, tag="ps_chunk")
            for q in range(NQ):
                nc.tensor.matmul(
                    ps[:, q * 128:(q + 1) * 128],
                    xb[:, q, c * KC:(c + 1) * KC],
                    ident_sb,
                    is_transpose=True,
                    skip_group_check=True,
                )
            xT = xtpool.tile([KC, UNIT_B], BF16, tag="xT")
            if c % 2 == 0:
                nc.vector.tensor_copy(out=xT, in_=ps)
            else:
                nc.scalar.copy(out=xT, in_=ps)
            nc.tensor.matmul(
                s1,
                w_sb[:, c, :],
                xT,
                start=(c == 0),
                stop=(c == NCHUNK - 1),
            )

        # log-space scan over the 40 positions (rows 1..40 of s1; row 0 = j
        # passes through Ln harmlessly and is never read downstream)
        l_sb = lpool.tile([41, UNIT_B], BF16, tag="l_sb")
        nc.scalar.activation(out=l_sb, in_=s1,
                             func=mybir.ActivationFunctionType.Ln,
                             scale=ln_scale)
        cum = ps_cum.tile([L, UNIT_B], F32, tag="cum")
        nc.tensor.matmul(cum, tri_sb, l_sb, start=True, stop=True)
        e_sb = epool.tile([L, UNIT_B], BF16, tag="e_sb")
        nc.scalar.activation(out=e_sb, in_=cum,
                             func=mybir.ActivationFunctionType.Exp,
                             bias=expbias_sb)

        v_ps = ps_vj.tile([1, UNIT_B], F32, tag="vj")
        nc.tensor.matmul(v_ps, colv_sb, e_sb, start=True, stop=True)

        stage = stpool.tile([1, UNIT_B, 2], F32, tag="stage")
        nc.vector.tensor_copy(out=stage[:, :, 0], in_=v_ps)
        nc.vector.tensor_copy(out=stage[:, :, 1], in_=s1[0:1, :])

        nc.sync.dma_start(
            out=out_view[u].rearrange("(p f) -> p f", p=1),
            in_=stage.rearrange("p n c -> p (n c)"),
        )


_PROGRAM_CACHE = {}


def _get_program(b_shard: int):
    if b_shard not in _PROGRAM_CACHE:
        nc = bacc.Bacc("TRN2", debug=False, enable_asserts=False,
                       num_devices=N_CORES)
        x_d = nc.dram_tensor("x", [b_shard, L, A], F32, kind="ExternalInput")
        v_d = nc.dram_tensor("v_germline_aa_onehot", [L, A], F32,
                             kind="ExternalInput")
        j_d = nc.dram_tensor("j_germline_aa_onehot", [L, A], F32,
                             kind="ExternalInput")
        out_d = nc.dram_tensor("out", [b_shard, 2], F32, kind="ExternalOutput")
        with tile.TileContext(nc) as tc:
            with ExitStack() as ctx:
                emit_kernel(ctx, tc, x_d.ap(), v_d.ap(), j_d.ap(), out_d.ap())
        nc.compile()
        _PROGRAM_CACHE[b_shard] = nc
    return _PROGRAM_CACHE[b_shard]


def kernel(x, v_germline_aa_onehot, j_germline_aa_onehot, _trace=False):
    x = np.ascontiguousarray(np.asarray(x, dtype=np.float32))
    v = np.ascontiguousarray(np.asarray(v_germline_aa_onehot, dtype=np.float32))
    j = np.ascontiguousarray(np.asarray(j_germline_aa_onehot, dtype=np.float32))
    B = x.shape[0]
    assert B % N_CORES == 0
    b_shard = B // N_CORES

    nc = _get_program(b_shard)
    in_maps = [
        {
            "x": x[i * b_shard:(i + 1) * b_shard],
            "v_germline_aa_onehot": v,
            "j_germline_aa_onehot": j,
        }
        for i in range(N_CORES)
    ]
    res = run_bass_kernel_spmd(nc, in_maps, core_ids=list(range(N_CORES)),
                               trace=_trace)
    out = np.concatenate([r["out"] for r in res.results], axis=0)
    if _trace:
        return out, res
    return out


# revision 10
# speedup vs baseline: 1900.9033x; 1900.9033x over previous
"""Trainium2 Bass kernel for nn_ContiguousMatch.

Math (per batch row b):
    v_match[b,l] = sum_a x[b,l,a] * v[l,a]            # [B, 40]
    v_score[b]   = sum_l cumprod_l(v_match[b,:])       # [B]
    j_score[b]   = sum_{l,a} x[b,l,a] * j[l,a]         # [B]
    out[b] = (v_score[b], j_score[b])                  # [B, 2]

Strategy (pure data parallel over 8 cores, B_shard = 16384 per core):
  All heavy math runs on the TensorEngine in bf16.  x tiles [128b, 840]
  stream in through a casting SWDGE DMA (fp32 HBM -> bf16 SBUF, zero
  compute cost), get transposed on-chip by PE into [120k, 512b] chunks
  (7 chunks cover the 840 = 40*21 feature axis), are copied PSUM->SBUF
  by DVE/ACT, then a block-diagonal germline matrix W [840, 41] (col l
  holds v[l,:] in rows l*21..l*21+20, col 40 holds j.flatten()) reduces
  them in 7 accumulating matmuls to S = [41, 512] fp32: rows 0..39 =
  v_match^T, row 40 = j_score (taken exactly from here).
  The cumprod-scan runs in log space: Ln on ACT with a constant scale
  e^{-c} (keeps bf16 logs centered near 0), cumulative sum as a matmul
  with a triangular ones matrix, Exp on ACT with a per-position bias
  (m+1)*c undoing the shift, and the sum over l as a ones-vector matmul.
  Results land interleaved in SBUF and stream out as one contiguous 4KB
  DMA per 512 rows.
"""

import numpy as np
import ml_dtypes
from contextlib import ExitStack

import concourse.bass as bass
import concourse.bacc as bacc
import concourse.tile as tile
from concourse import mybir
from concourse.bass_utils import run_bass_kernel_spmd

F32 = mybir.dt.float32
BF16 = mybir.dt.bfloat16
F16 = mybir.dt.float16
BF16_NP = ml_dtypes.bfloat16
F16_NP = np.float16

N_CORES = 8
B_FULL = 131072
L, A = 40, 21
KA = L * A            # 840
NCHUNK = 7
KC = KA // NCHUNK     # 120
UNIT_B = 512          # batch rows handled per unit
NQ = UNIT_B // 128    # 4 sub-tiles of 128 rows

# log-space centering constants (numerical conditioning only — the shift is
# exactly undone by the Exp bias, so correctness holds for any input stats)
C_V = 1.6582280766542983  # ~ log(E[v_match]) = log(21/4)


def _np_consts():
    ident = np.eye(128, dtype=F16_NP)
    # W col 0 = j, cols 1..40 = v one-hot blocks (j first so j_score lands
    # on PSUM partition 0, which keeps the extraction copy 32-aligned)
    mask = np.zeros((KC, NCHUNK, 41), F16_NP)
    for c in range(NCHUNK):
        for p in range(KC):
            l = (c * KC + p) // A
            mask[p, c, 1 + l] = 1.0
    # cum[m] = sum of log rows 1..m+1 (row 0 = j row is skipped)
    tri = np.zeros((41, L), F16_NP)
    for m in range(L):
        tri[1: m + 2, m] = 1.0
    colv = np.ones((L, 1), BF16_NP)
    expbias = ((np.arange(L, dtype=np.float64) + 1.0) * C_V).astype(np.float32)
    expbias = expbias.reshape(L, 1)
    return ident, mask, tri, colv, expbias


def emit_kernel(ctx: ExitStack, tc: tile.TileContext,
                x_ap, v_ap, j_ap, out_ap):
    """Emit the per-core program. x_ap [B,L,A], v/j [L,A], out [B,2]."""
    nc = tc.nc
    B = x_ap.shape[0]
    assert B % UNIT_B == 0
    n_units = B // UNIT_B

    ident_np, mask_np, tri_np, colv_np, expbias_np = _np_consts()
    ident_d = nc.inline_tensor(ident_np, name="c_ident")
    mask_d = nc.inline_tensor(mask_np, name="c_mask")
    tri_d = nc.inline_tensor(tri_np, name="c_tri")
    colv_d = nc.inline_tensor(colv_np, name="c_colv")
    expbias_d = nc.inline_tensor(expbias_np, name="c_expbias")

    singles = ctx.enter_context(tc.tile_pool(name="singles", bufs=1))
    xpool = ctx.enter_context(tc.tile_pool(name="xpool", bufs=3))
    xtpool = ctx.enter_context(tc.tile_pool(name="xtpool", bufs=4))
    lpool = ctx.enter_context(tc.tile_pool(name="lpool", bufs=2))
    epool = ctx.enter_context(tc.tile_pool(name="epool", bufs=2))
    stpool = ctx.enter_context(tc.tile_pool(name="stpool", bufs=3))
    ps_chunk = ctx.enter_context(tc.tile_pool(name="ps_chunk", bufs=3, space="PSUM"))
    ps_s1 = ctx.enter_context(tc.tile_pool(name="ps_s1", bufs=2, space="PSUM"))
    ps_cum = ctx.enter_context(tc.tile_pool(name="ps_cum", bufs=1, space="PSUM"))
    ps_vj = ctx.enter_context(tc.tile_pool(name="ps_vj", bufs=2, space="PSUM"))

    # ---- constants into SBUF ----
    ident_sb = singles.tile([128, 128], F16)
    nc.sync.dma_start(out=ident_sb, in_=ident_d.ap())
    # mask goes through a DVE staging copy so the broadcast-multiply below
    # (TensorScalarPtr encoding: single wait slot) needs no cross-engine waits
    mask_stg = singles.tile([KC, NCHUNK, 41], F16)
    nc.sync.dma_start(out=mask_stg, in_=mask_d.ap())
    mask_sb = singles.tile([KC, NCHUNK, 41], F16)
    nc.vector.tensor_copy(out=mask_sb, in_=mask_stg)
    tri_sb = singles.tile([41, L], F16)
    nc.sync.dma_start(out=tri_sb, in_=tri_d.ap())
    colv_sb = singles.tile([L, 1], BF16)
    nc.sync.dma_start(out=colv_sb, in_=colv_d.ap())
    expbias_sb = singles.tile([L, 1], F32)
    nc.sync.dma_start(out=expbias_sb, in_=expbias_d.ap())

    # ---- germline -> W [KC, NCHUNK, 41] (bf16) ----
    # load v/j flattened as [7, 120] (partition = chunk), PE-transpose to [120, 7]
    vg_sb = singles.tile([NCHUNK, KC], F16)
    nc.gpsimd.dma_start(out=vg_sb,
                        in_=v_ap.flatten().rearrange("(c k) -> c k", c=NCHUNK))
    jg_sb = singles.tile([NCHUNK, KC], F16)
    nc.gpsimd.dma_start(out=jg_sb,
                        in_=j_ap.flatten().rearrange("(c k) -> c k", c=NCHUNK))

    v_col = singles.tile([KC, NCHUNK], F32)
    j_col = singles.tile([KC, NCHUNK], F32)
    for src, dst in ((vg_sb, v_col), (jg_sb, j_col)):
        tr_ps = ps_chunk.tile([KC, UNIT_B], F16, tag="ps_chunk")
        nc.tensor.transpose(tr_ps[:, :NCHUNK], src, ident_sb[:NCHUNK, :NCHUNK])
        nc.vector.tensor_copy(out=dst, in_=tr_ps[:, :NCHUNK])

    w_sb = singles.tile([KC, NCHUNK, 41], F16)
    nc.vector.tensor_mul(w_sb[:, :, 1:41], mask_sb[:, :, 1:41],
                         v_col.to_broadcast([KC, NCHUNK, L]))
    nc.vector.tensor_copy(out=w_sb[:, :, 0], in_=j_col)

    # x viewed as [u, p, q, 840]
    x_view = x_ap.rearrange("(u q p) l a -> u p q (l a)", q=NQ, p=128)
    out_view = out_ap.rearrange("(u f) c -> u (f c)", f=UNIT_B)

    ln_scale = float(np.exp(-C_V))

    for u in range(n_units):
        xb = xpool.tile([128, NQ, KA], F16, tag="xb")
        nc.gpsimd.dma_start(out=xb, in_=x_view[u])  # casting DMA fp32->bf16

        s1 = ps_s1.tile([41, UNIT_B], F32, tag="s1")
        for c in range(NCHUNK):
            ps = ps_chunk.tile([KC, UNIT_B], F16, tag="ps_chunk")
            for q in range(NQ):
                nc.tensor.matmul(
                    ps[:, q * 128:(q + 1) * 128],
                    xb[:, q, c * KC:(c + 1) * KC],
                    ident_sb,
                    is_transpose=True,
                    skip_group_check=True,
                )
            xT = xtpool.tile([KC, UNIT_B], F16, tag="xT")
            if c % 2 == 0:
                nc.vector.tensor_copy(out=xT, in_=ps)
            else:
                nc.scalar.copy(out=xT, in_=ps)
            nc.tensor.matmul(
                s1,
                w_sb[:, c, :],
                xT,
                start=(c == 0),
                stop=(c == NCHUNK - 1),
            )

        # log-space scan over the 40 positions (rows 1..40 of s1; row 0 = j
        # passes through Ln harmlessly and is never read downstream)
        l_sb = lpool.tile([41, UNIT_B], F16, tag="l_sb")
        nc.scalar.activation(out=l_sb, in_=s1,
                             func=mybir.ActivationFunctionType.Ln,
                             scale=ln_scale)
        cum = ps_cum.tile([L, UNIT_B], F32, tag="cum")
        nc.tensor.matmul(cum, tri_sb, l_sb, start=True, stop=True)
        e_sb = epool.tile([L, UNIT_B], BF16, tag="e_sb")
        nc.scalar.activation(out=e_sb, in_=cum,
                             func=mybir.ActivationFunctionType.Exp,
                             bias=expbias_sb)

        v_ps = ps_vj.tile([1, UNIT_B], F32, tag="vj")
        nc.tensor.matmul(v_ps, colv_sb, e_sb, start=True, stop=True)

        stage = stpool.tile([1, UNIT_B, 2], F32, tag="stage")
        nc.vector.tensor_copy(out=stage[:, :, 0], in_=v_ps)
        nc.vector.tensor_copy(out=stage[:, :, 1], in_=s1[0:1, :])

        nc.sync.dma_start(
            out=out_view[u].rearrange("(p f) -> p f", p=1),
            in_=stage.rearrange("p n c -> p (n c)"),
        )


_PROGRAM_CACHE = {}


def _get_program(b_shard: int):
    if b_shard not in _PROGRAM_CACHE:
        nc = bacc.Bacc("TRN2", debug=False, enable_asserts=False,
                       num_devices=N_CORES)
        x_d = nc.dram_tensor("x", [b_shard, L, A], F32, kind="ExternalInput")
        v_d = nc.dram_tensor("v_germline_aa_onehot", [L, A], F32,
                             kind="ExternalInput")
        j_d = nc.dram_tensor("j_germline_aa_onehot", [L, A], F32,
                             kind="ExternalInput")
        out_d = nc.dram_tensor("out", [b_shard, 2], F32, kind="ExternalOutput")
        with tile.TileContext(nc) as tc:
            with ExitStack() as ctx:
                emit_kernel(ctx, tc, x_d.ap(), v_d.ap(), j_d.ap(), out_d.ap())
        nc.compile()
        _PROGRAM_CACHE[b_shard] = nc
    return _PROGRAM_CACHE[b_shard]


def kernel(x, v_germline_aa_onehot, j_germline_aa_onehot, _trace=False):
    x = np.ascontiguousarray(np.asarray(x, dtype=np.float32))
    v = np.ascontiguousarray(np.asarray(v_germline_aa_onehot, dtype=np.float32))
    j = np.ascontiguousarray(np.asarray(j_germline_aa_onehot, dtype=np.float32))
    B = x.shape[0]
    assert B % N_CORES == 0
    b_shard = B // N_CORES

    nc = _get_program(b_shard)
    in_maps = [
        {
            "x": x[i * b_shard:(i + 1) * b_shard],
            "v_germline_aa_onehot": v,
            "j_germline_aa_onehot": j,
        }
        for i in range(N_CORES)
    ]
    res = run_bass_kernel_spmd(nc, in_maps, core_ids=list(range(N_CORES)),
                               trace=_trace)
    out = np.concatenate([r["out"] for r in res.results], axis=0)
    if _trace:
        return out, res
    return out


# revision 15
# speedup vs baseline: 49177.8574x; 25.8708x over previous
"""Trainium2 Bass kernel for nn_ContiguousMatch.

Math (per batch row b):
    v_match[b,l] = sum_a x[b,l,a] * v[l,a]            # [B, 40]
    v_score[b]   = sum_l cumprod_l(v_match[b,:])       # [B]
    j_score[b]   = sum_{l,a} x[b,l,a] * j[l,a]         # [B]
    out[b] = (v_score[b], j_score[b])                  # [B, 2]

Strategy (pure data parallel over 8 cores, B_shard = 16384 per core):
  All heavy math runs on the TensorEngine in bf16.  x tiles [128b, 840]
  stream in through a casting SWDGE DMA (fp32 HBM -> bf16 SBUF, zero
  compute cost), get transposed on-chip by PE into [120k, 512b] chunks
  (7 chunks cover the 840 = 40*21 feature axis), are copied PSUM->SBUF
  by DVE/ACT, then a block-diagonal germline matrix W [840, 41] (col l
  holds v[l,:] in rows l*21..l*21+20, col 40 holds j.flatten()) reduces
  them in 7 accumulating matmuls to S = [41, 512] fp32: rows 0..39 =
  v_match^T, row 40 = j_score (taken exactly from here).
  The cumprod-scan runs in log space: Ln on ACT with a constant scale
  e^{-c} (keeps bf16 logs centered near 0), cumulative sum as a matmul
  with a triangular ones matrix, Exp on ACT with a per-position bias
  (m+1)*c undoing the shift, and the sum over l as a ones-vector matmul.
  Results land interleaved in SBUF and stream out as one contiguous 4KB
  DMA per 512 rows.
"""

import numpy as np
import ml_dtypes
from contextlib import ExitStack

import concourse.bass as bass
import concourse.bacc as bacc
import concourse.tile as tile
from concourse import mybir
from concourse.bass_utils import run_bass_kernel_spmd

F32 = mybir.dt.float32
BF16 = mybir.dt.bfloat16
F16 = mybir.dt.float16
BF16_NP = ml_dtypes.bfloat16
F16_NP = np.float16

N_CORES = 8
B_FULL = 131072
L, A = 40, 21
KA = L * A            # 840
NCHUNK = 7
KC = KA // NCHUNK     # 120
UNIT_B = 512          # batch rows handled per unit
NQ = UNIT_B // 128    # 4 sub-tiles of 128 rows

# log-space centering constants (numerical conditioning only — the shift is
# exactly undone by the Exp bias, so correctness holds for any input stats)
C_V = 1.6582280766542983  # ~ log(E[v_match]) = log(21/4)


def _np_consts():
    ident = np.eye(128, dtype=F16_NP)
    # W col 0 = j, cols 1..40 = v one-hot blocks (j first so j_score lands
    # on PSUM partition 0, which keeps the extraction copy 32-aligned)
    mask = np.zeros((KC, NCHUNK, 41), F16_NP)
    for c in range(NCHUNK):
        for p in range(KC):
            l = (c * KC + p) // A
            mask[p, c, 1 + l] = 1.0
    # cum[m] = sum of log rows 1..m+1 (row 0 = j row is skipped)
    tri = np.zeros((41, L), F16_NP)
    for m in range(L):
        tri[1: m + 2, m] = 1.0
    colv = np.ones((L, 1), BF16_NP)
    expbias = ((np.arange(L, dtype=np.float64) + 1.0) * C_V).astype(np.float32)
    expbias = expbias.reshape(L, 1)
    return ident, mask, tri, colv, expbias


def emit_kernel(ctx: ExitStack, tc: tile.TileContext,
                x_ap, v_ap, j_ap, out_ap, repeat=1):
    """Emit the per-core program. x_ap [B,L,A], v/j [L,A], out [B,2]."""
    nc = tc.nc
    B = x_ap.shape[0]
    assert B % UNIT_B == 0
    n_units = B // UNIT_B

    ident_np, mask_np, tri_np, colv_np, expbias_np = _np_consts()
    ident_d = nc.inline_tensor(ident_np, name="c_ident")
    mask_d = nc.inline_tensor(mask_np, name="c_mask")
    tri_d = nc.inline_tensor(tri_np, name="c_tri")
    colv_d = nc.inline_tensor(colv_np, name="c_colv")
    expbias_d = nc.inline_tensor(expbias_np, name="c_expbias")

    singles = ctx.enter_context(tc.tile_pool(name="singles", bufs=1))
    xpool = ctx.enter_context(tc.tile_pool(name="xpool", bufs=3))
    xtpool = ctx.enter_context(tc.tile_pool(name="xtpool", bufs=4))
    lpool = ctx.enter_context(tc.tile_pool(name="lpool", bufs=2))
    epool = ctx.enter_context(tc.tile_pool(name="epool", bufs=2))
    stpool = ctx.enter_context(tc.tile_pool(name="stpool", bufs=3))
    ps_chunk = ctx.enter_context(tc.tile_pool(name="ps_chunk", bufs=3, space="PSUM"))
    ps_s1 = ctx.enter_context(tc.tile_pool(name="ps_s1", bufs=2, space="PSUM"))
    ps_cum = ctx.enter_context(tc.tile_pool(name="ps_cum", bufs=1, space="PSUM"))
    ps_vj = ctx.enter_context(tc.tile_pool(name="ps_vj", bufs=2, space="PSUM"))

    # ---- constants into SBUF ----
    ident_sb = singles.tile([128, 128], F16)
    nc.sync.dma_start(out=ident_sb, in_=ident_d.ap())
    # mask goes through a DVE staging copy so the broadcast-multiply below
    # (TensorScalarPtr encoding: single wait slot) needs no cross-engine waits
    mask_stg = singles.tile([KC, NCHUNK, 41], F16)
    nc.sync.dma_start(out=mask_stg, in_=mask_d.ap())
    mask_sb = singles.tile([KC, NCHUNK, 41], F16)
    nc.vector.tensor_copy(out=mask_sb, in_=mask_stg)
    tri_sb = singles.tile([41, L], F16)
    nc.sync.dma_start(out=tri_sb, in_=tri_d.ap())
    colv_sb = singles.tile([L, 1], BF16)
    nc.sync.dma_start(out=colv_sb, in_=colv_d.ap())
    expbias_sb = singles.tile([L, 1], F32)
    nc.sync.dma_start(out=expbias_sb, in_=expbias_d.ap())

    # ---- germline -> W [KC, NCHUNK, 41] (bf16) ----
    # load v/j flattened as [7, 120] (partition = chunk), PE-transpose to [120, 7]
    vg_sb = singles.tile([NCHUNK, KC], F16)
    nc.gpsimd.dma_start(out=vg_sb,
                        in_=v_ap.flatten().rearrange("(c k) -> c k", c=NCHUNK))
    jg_sb = singles.tile([NCHUNK, KC], F16)
    nc.gpsimd.dma_start(out=jg_sb,
                        in_=j_ap.flatten().rearrange("(c k) -> c k", c=NCHUNK))

    v_col = singles.tile([KC, NCHUNK], F32)
    j_col = singles.tile([KC, NCHUNK], F32)
    for src, dst in ((vg_sb, v_col), (jg_sb, j_col)):
        tr_ps = ps_chunk.tile([KC, UNIT_B], F16, tag="ps_chunk")
        nc.tensor.transpose(tr_ps[:, :NCHUNK], src, ident_sb[:NCHUNK, :NCHUNK])
        nc.vector.tensor_copy(out=dst, in_=tr_ps[:, :NCHUNK])

    w_sb = singles.tile([KC, NCHUNK, 41], F16)
    nc.vector.tensor_mul(w_sb[:, :, 1:41], mask_sb[:, :, 1:41],
                         v_col.to_broadcast([KC, NCHUNK, L]))
    nc.vector.tensor_copy(out=w_sb[:, :, 0], in_=j_col)

    # x viewed as [u, p, q, 840]
    x_view = x_ap.rearrange("(u q p) l a -> u p q (l a)", q=NQ, p=128)
    out_view = out_ap.rearrange("(u f) c -> u (f c)", f=UNIT_B)

    ln_scale = float(np.exp(-C_V))

    for u in [u for _ in range(repeat) for u in range(n_units)]:
        xb = xpool.tile([128, NQ, KA], F16, tag="xb")
        nc.gpsimd.dma_start(out=xb, in_=x_view[u])  # casting DMA fp32->fp16

        s1 = ps_s1.tile([41, UNIT_B], F32, tag="s1")
        for c in range(NCHUNK):
            ps = ps_chunk.tile([KC, UNIT_B], F16, tag="ps_chunk")
            for q in range(NQ):
                nc.tensor.matmul(
                    ps[:, q * 128:(q + 1) * 128],
                    xb[:, q, c * KC:(c + 1) * KC],
                    ident_sb,
                    is_transpose=True,
                    skip_group_check=True,
                )
            xT = xtpool.tile([KC, UNIT_B], F16, tag="xT")
            if c % 2 == 0:
                nc.vector.tensor_copy(out=xT, in_=ps)
            else:
                nc.scalar.copy(out=xT, in_=ps)
            nc.tensor.matmul(
                s1,
                w_sb[:, c, :],
                xT,
                start=(c == 0),
                stop=(c == NCHUNK - 1),
            )

        # log-space scan over the 40 positions (rows 1..40 of s1; row 0 = j
        # passes through Ln harmlessly and is never read downstream)
        l_sb = lpool.tile([41, UNIT_B], F16, tag="l_sb")
        nc.scalar.activation(out=l_sb, in_=s1,
                             func=mybir.ActivationFunctionType.Ln,
                             scale=ln_scale)
        cum = ps_cum.tile([L, UNIT_B], F32, tag="cum")
        nc.tensor.matmul(cum, tri_sb, l_sb, start=True, stop=True)
        e_sb = epool.tile([L, UNIT_B], BF16, tag="e_sb")
        nc.scalar.activation(out=e_sb, in_=cum,
                             func=mybir.ActivationFunctionType.Exp,
                             bias=expbias_sb)

        v_ps = ps_vj.tile([1, UNIT_B], F32, tag="vj")
        nc.tensor.matmul(v_ps, colv_sb, e_sb, start=True, stop=True)

        stage = stpool.tile([1, UNIT_B, 2], F32, tag="stage")
        nc.vector.tensor_copy(out=stage[:, :, 0], in_=v_ps)
        nc.vector.tensor_copy(out=stage[:, :, 1], in_=s1[0:1, :])

        nc.sync.dma_start(
            out=out_view[u].rearrange("(p f) -> p f", p=1),
            in_=stage.rearrange("p n c -> p (n c)"),
        )


_PROGRAM_CACHE = {}


def _get_program(b_shard: int, repeat: int = 1):
    if (b_shard, repeat) not in _PROGRAM_CACHE:
        nc = bacc.Bacc("TRN2", debug=False, enable_asserts=False,
                       num_devices=N_CORES)
        x_d = nc.dram_tensor("x", [b_shard, L, A], F32, kind="ExternalInput")
        v_d = nc.dram_tensor("v_germline_aa_onehot", [L, A], F32,
                             kind="ExternalInput")
        j_d = nc.dram_tensor("j_germline_aa_onehot", [L, A], F32,
                             kind="ExternalInput")
        out_d = nc.dram_tensor("out", [b_shard, 2], F32, kind="ExternalOutput")
        with tile.TileContext(nc) as tc:
            with ExitStack() as ctx:
                emit_kernel(ctx, tc, x_d.ap(), v_d.ap(), j_d.ap(), out_d.ap(),
                            repeat=repeat)
        nc.compile()
        _PROGRAM_CACHE[(b_shard, repeat)] = nc
    return _PROGRAM_CACHE[(b_shard, repeat)]


def kernel(x, v_germline_aa_onehot, j_germline_aa_onehot, _trace=False):
    x = np.ascontiguousarray(np.asarray(x, dtype=np.float32))
    v = np.ascontiguousarray(np.asarray(v_germline_aa_onehot, dtype=np.float32))
    j = np.ascontiguousarray(np.asarray(j_germline_aa_onehot, dtype=np.float32))
    B = x.shape[0]
    assert B % N_CORES == 0
    b_shard = B // N_CORES

    nc = _get_program(b_shard)
    in_maps = [
        {
            "x": x[i * b_shard:(i + 1) * b_shard],
            "v_germline_aa_onehot": v,
            "j_germline_aa_onehot": j,
        }
        for i in range(N_CORES)
    ]
    res = run_bass_kernel_spmd(nc, in_maps, core_ids=list(range(N_CORES)),
                               trace=_trace)
    out = np.concatenate([r["out"] for r in res.results], axis=0)
    if _trace:
        return out, res
    return out
